# revision 8
# baseline (speedup 1.0000x reference)
"""DepthLoss kernel for 8 Trainium2 NeuronCores — deduplicated gather.

reference:
    rows/cols/d = rdepth[...,0/1/2]; mask = d>0
    vals = output[b, 0, rows, cols]
    loss = sum(mask * |vals - d|) / max(count(mask), 1)   (0 if count==0)

Strategy: data-parallel over batch (4 planes/core). The host packs the
VALID samples (d>0; invalid ones contribute nothing) into gather "slots"
of up to two same-row samples each — samples that share a 256B image row
are paired so the row is fetched once. Slots are dealt round-robin across
partitions with paired slots first, so the second-sample select layer
covers only a fixed low-column block. Per batch this shrinks the SWDGE
gather from 16384 rows to <= 8704 (measured ~8300), a ~2x byte cut.

Device per iteration: ONE idx dma + ONE aux dma; 36 dma_gather calls;
per batch 3 fused one-hot select passes (two blocks for sample layer A,
one block for layer B) + per-row reduce + masked |v - d| partial sums.
Host combines the 8 cores' [128, 2] partials and divides.

Slot (p, jj) of a batch is gather position i = 128*jj + p; its row id is
stored (replicated across the 8 gpsimd stripes) at idx tile
[p%16, 64*(jj//8) + 8*(jj%8) + p//16]. Pad positions get idx -1
(negative indices at the end of a call are skipped by the gather).
"""

import numpy as np

import concourse.bacc as bacc
import concourse.mybir as mybir
import concourse.tile as tile
from concourse import library_config
from concourse.bass_utils import run_bass_kernel_spmd

# --- custom DVE op registration (idempotent) -------------------------------
import concourse.dve_ops as _dvo
from concourse.dve_spec import (
    Spec as _Spec, Src0 as _S0, Src1 as _S1, eq as _eq, Idx as _Idx,
)


def _ref_selmm(in0, in1, c0, c1, c2):
    P_ = in0.shape[0]
    x0 = in0.reshape(P_, -1).astype(np.float32)
    x1 = np.broadcast_to(in1, in0.shape).reshape(P_, -1).astype(np.float32)
    k = np.arange(x0.shape[1], dtype=np.float32)[None, :]
    return ((x1 == k) * x0).astype(np.float32)


def _register_selmm():
    name = "SEL_MASK_MUL_ANT"
    if name in _dvo._SUB_OPCODE_FOR_NAME:
        return next(op for op in _dvo.OPS if op.name == name)
    spec = _Spec(body=_eq(_S1, _Idx) * _S0, reference=_ref_selmm)
    op = _dvo.DveOp(
        name, spec, subdim=False,
        uops_sha={"v3": "8167b76bec34326c", "v4": "12842eb32a8347cf"},
    )
    row = max(_dvo._SUB_OPCODE_FOR_NAME.values()) + 1
    assert row < 0x20
    _dvo.OPS.append(op)
    _dvo._SUB_OPCODE_FOR_NAME[name] = row
    _dvo.CUSTOM_DVE_SPECS[name] = op.spec
    return op


SEL_MASK_MUL = _register_selmm()

B, N, H, W = 32, 16384, 768, 1024
NCORES = 8
BPC = B // NCORES          # batches (planes) per core = 4
P = 128
PLANE = H * W              # 786432
E = 64                     # gathered row length (f32) = 256 B
RT = PLANE // E            # rows per plane table = 12288
JJ2 = 72                   # gather slots per partition per batch
NS2 = P * JJ2              # slots per batch = 8704 (>= ~8300 measured)
U2 = NS2 // 16             # idx cols per batch = 576
J2B = 32                   # max paired-slot cols per partition
# select passes: (block_start, block_len, layer) — layer 0 = sample A,
# layer 1 = sample B (paired slots only, low cols)
PASSES = ((0, 36, 0), (36, 36, 0), (0, J2B, 1))
PASSLEN = sum(p[1] for p in PASSES)      # 100 table cols per batch
# gather chunks: (num_idxs, idx cols, G cols)
CHUNKS = [(1024, 64, 8)] * 9
BIG = 1048576.0
F32 = mybir.dt.float32
I16 = mybir.dt.int16
Alu = mybir.AluOpType
AX = mybir.AxisListType


def build(n_iters=1, init_unused=True):
    nc = bacc.Bacc(
        "TRN2", target_bir_lowering=False, debug=False,
        num_swdge_queues=4, dynamic_dma_scratch_size=32768,
    )

    img = nc.dram_tensor("img", [BPC * RT, E], F32, kind="ExternalInput")
    idx = nc.dram_tensor("idx", [P, BPC * U2], I16, kind="ExternalInput")
    aux = nc.dram_tensor("aux", [P, 2 * BPC * PASSLEN], F32,
                         kind="ExternalInput")
    out = nc.dram_tensor("out", [P, 2], F32, kind="ExternalOutput")

    with tile.TileContext(nc) as tc:
        with (
            tc.tile_pool(name="acc", bufs=1) as acc,
            tc.tile_pool(name="inp", bufs=2) as inp,
            tc.tile_pool(name="big", bufs=3) as big,
            tc.psum_pool(name="wp", bufs=1) as wp,
            tc.tile_pool(name="sm", bufs=2) as sm,
        ):
            nc.gpsimd.load_library(library_config.mlp)

            for _ in range(n_iters):
                idx16 = inp.tile([P, BPC * U2], I16, tag="idx16")
                nc.sync.dma_start(out=idx16[:], in_=idx[:, :])
                auxt = inp.tile([P, 2 * BPC * PASSLEN], F32, tag="auxt")
                nc.sync.dma_start(out=auxt[:], in_=aux[:, :])

                NP = len(PASSES)
                lc = acc.tile([P, BPC * NP], F32, tag="lc")
                cc2 = acc.tile([P, BPC * NP], F32, tag="cc2")
                gs = []
                for b in range(BPC):
                    g = big.tile([P, JJ2 * E], F32, tag="G")
                    g3 = g[:].rearrange("p (j e) -> p j e", e=E)
                    gs.append(g3)
                    uoff = b * U2
                    joff = 0
                    for k, (ci, cu, cj) in enumerate(CHUNKS):
                        nc.gpsimd.dma_gather(
                            g3[:, joff : joff + cj, :],
                            img[b * RT : (b + 1) * RT, :],
                            idx16[:, uoff : uoff + cu],
                            ci,
                            ci,
                            E,
                            single_packet=False,
                            queue_num=k % 4,
                        )
                        uoff += cu
                        joff += cj

                for b in range(BPC):
                    g3 = gs[b]
                    toff = b * PASSLEN
                    for pidx, (bs, bl, _layer) in enumerate(PASSES):
                        gt = auxt[:, toff : toff + bl]
                        ds = auxt[:, BPC * PASSLEN + toff : BPC * PASSLEN
                                  + toff + bl]
                        toff += bl

                        msel = sm.tile([P, bl], F32, tag=f"msel{pidx}")
                        nc.vector.tensor_scalar(
                            out=msel[:], in0=ds, scalar1=0.0, scalar2=None,
                            op0=Alu.is_gt,
                        )
                        w = wp.tile([P, 36 * E], F32, tag="W")
                        w3 = w[:].rearrange("p (j e) -> p j e", e=E)[
                            :, :bl, :
                        ]
                        vsel = sm.tile([P, bl], F32, tag=f"vsel{pidx}")
                        nc.vector._custom_dve(
                            SEL_MASK_MUL,
                            out=w3[:, :, :],
                            in0=g3[:, bs : bs + bl, :],
                            in1=gt.unsqueeze(2).to_broadcast([P, bl, E]),
                        )
                        nc.vector.tensor_reduce(
                            out=vsel[:], in_=w3[:, :, :], axis=AX.X,
                            op=Alu.add,
                        )
                        diff = sm.tile([P, bl], F32, tag=f"diff{pidx}")
                        nc.vector.tensor_tensor(
                            out=diff[:], in0=vsel[:], in1=ds, op=Alu.subtract
                        )
                        nc.vector.tensor_tensor(
                            out=diff[:], in0=diff[:], in1=msel[:], op=Alu.mult
                        )
                        col = b * NP + pidx
                        nc.vector.tensor_reduce(
                            out=lc[:, col : col + 1], in_=diff[:], axis=AX.X,
                            op=Alu.add, apply_absolute_value=True,
                        )
                        nc.vector.tensor_reduce(
                            out=cc2[:, col : col + 1], in_=msel[:], axis=AX.X,
                            op=Alu.add,
                        )

                losscnt = acc.tile([P, 2], F32, tag="losscnt")
                nc.vector.tensor_reduce(
                    out=losscnt[:, 0:1], in_=lc[:], axis=AX.X, op=Alu.add
                )
                nc.vector.tensor_reduce(
                    out=losscnt[:, 1:2], in_=cc2[:], axis=AX.X, op=Alu.add
                )
                nc.sync.dma_start(out=out[:, :], in_=losscnt[:])

    nc.compile()
    return nc


_NC = None


def _get_nc():
    global _NC
    if _NC is None:
        _NC = build(init_unused=False)
    return _NC


def _pack_batch(rowid, cmod, d):
    """Pack one plane's valid samples into slots of <=2 same-row samples.

    Returns idx_wrapped [16, U2] int16, gt [128, PASSLEN], dp [128, PASSLEN].
    """
    valid = np.where(d > 0)[0]
    rv = rowid[valid]
    order = np.argsort(rv, kind="stable")
    rs = rv[order]
    uniq, start, counts = np.unique(rs, return_index=True, return_counts=True)
    nsl = (counts + 1) // 2
    tot = int(nsl.sum())
    assert tot <= NS2, tot
    run = np.repeat(np.arange(uniq.size), nsl)
    cum = np.cumsum(nsl) - nsl
    mloc = np.arange(tot) - np.repeat(cum, nsl)
    ia = np.repeat(start, nsl) + 2 * mloc
    ib = ia + 1
    hasb = ib < np.repeat(start + counts, nsl)
    sA = valid[order[ia]]
    sB = np.where(hasb, valid[order[np.minimum(ib, rs.size - 1)]], -1)
    rows = uniq[run]
    # paired slots first, then singles; pads at the very end
    srt = np.argsort(~hasb, kind="stable")
    rows, sA, sB = rows[srt], sA[srt], sB[srt]
    ndbl = int(hasb.sum())
    assert ndbl <= J2B * P, ndbl

    rows_p = np.zeros(NS2, np.int64)
    rows_p[:tot] = rows
    sA_p = np.full(NS2, -1, np.int64)
    sA_p[:tot] = sA
    sB_p = np.full(NS2, -1, np.int64)
    sB_p[:tot] = sB
    # position i -> slot (p, jj) = (i % 128, i // 128)
    row_slot = rows_p.reshape(JJ2, P).T      # [128, JJ2]
    sA_s = sA_p.reshape(JJ2, P).T
    sB_s = sB_p.reshape(JJ2, P).T

    # wrapped idx table
    jjs = np.arange(JJ2)
    ps = np.arange(P)
    u = 64 * (jjs[None, :] // 8) + 8 * (jjs[None, :] % 8) + ps[:, None] // 16
    idxw = np.zeros((16, U2), np.int16)
    idxw[ps[:, None] % 16, u] = row_slot.astype(np.int16)

    # per-pass select/mask tables
    gt = np.empty((P, PASSLEN), np.float32)
    dp = np.empty((P, PASSLEN), np.float32)
    off = 0
    for bs, bl, layer in PASSES:
        s = (sA_s if layer == 0 else sB_s)[:, bs : bs + bl]
        ok = s >= 0
        sc = np.minimum(np.maximum(s, 0), N - 1)
        cm = np.where(ok, cmod[sc], -BIG)
        gt[:, off : off + bl] = cm + 64.0 * np.arange(bl)[None, :]
        dp[:, off : off + bl] = np.where(ok, d[sc], -1.0)
        off += bl
    return idxw, gt, dp


def make_in_maps(output, rdepth):
    in_maps = []
    for c in range(NCORES):
        sl = slice(c * BPC, (c + 1) * BPC)
        img_c = np.ascontiguousarray(
            output[sl, 0], dtype=np.float32
        ).reshape(BPC * RT, E)

        rd = np.asarray(rdepth[sl], dtype=np.float32)  # [BPC, N, 3]
        rows = rd[..., 0].astype(np.int32)
        cols = rd[..., 1].astype(np.int32)
        d = rd[..., 2]
        pix = rows * W + cols
        rowid = pix >> 6
        cmod = (pix & 63).astype(np.float32)

        idx_c = np.empty((P, BPC * U2), np.int16)
        aux_c = np.empty((P, 2 * BPC * PASSLEN), np.float32)
        for b in range(BPC):
            idxw, gt, dp = _pack_batch(rowid[b], cmod[b], d[b])
            idx_c[:, b * U2 : (b + 1) * U2] = np.tile(idxw, (8, 1))
            aux_c[:, b * PASSLEN : (b + 1) * PASSLEN] = gt
            aux_c[:, (BPC + b) * PASSLEN : (BPC + b + 1) * PASSLEN] = dp

        in_maps.append({"img": img_c, "idx": idx_c, "aux": aux_c})
    return in_maps


def combine(results):
    partials = np.stack([r["out"] for r in results])  # [8, 128, 2]
    loss = partials[..., 0].astype(np.float64).sum()
    cnt = partials[..., 1].astype(np.float64).sum()
    val = loss / max(cnt, 1.0) if cnt > 0 else 0.0
    return np.asarray(val, dtype=np.float32)


def run(output, rdepth, **kw):
    res = run_bass_kernel_spmd(
        _get_nc(), make_in_maps(output, rdepth), list(range(NCORES)), **kw
    )
    return combine(res.results), res


def kernel(output, rdepth):
    return run(output, rdepth)[0]


# revision 9
# speedup vs baseline: 3.9846x; 3.9846x over previous
"""DepthLoss kernel for 8 Trainium2 NeuronCores — deduplicated gather.

reference:
    rows/cols/d = rdepth[...,0/1/2]; mask = d>0
    vals = output[b, 0, rows, cols]
    loss = sum(mask * |vals - d|) / max(count(mask), 1)   (0 if count==0)

Strategy: data-parallel over batch (4 planes/core). The host packs the
VALID samples (d>0; invalid ones contribute nothing) into gather "slots"
of up to two same-row samples each — samples that share a 256B image row
are paired so the row is fetched once. Slots are dealt round-robin across
partitions with paired slots first, so the second-sample select layer
covers only a fixed low-column block. Per batch this shrinks the SWDGE
gather from 16384 rows to <= 8704 (measured ~8300), a ~2x byte cut.

Device per iteration: ONE idx dma + ONE aux dma; 36 dma_gather calls;
per batch 3 fused one-hot select passes (two blocks for sample layer A,
one block for layer B) + per-row reduce + masked |v - d| partial sums.
Host combines the 8 cores' [128, 2] partials and divides.

Slot (p, jj) of a batch is gather position i = 128*jj + p; its row id is
stored (replicated across the 8 gpsimd stripes) at idx tile
[p%16, 64*(jj//8) + 8*(jj%8) + p//16]. Pad positions get idx -1
(negative indices at the end of a call are skipped by the gather).
"""

import numpy as np

import concourse.bacc as bacc
import concourse.mybir as mybir
import concourse.tile as tile
from concourse import library_config
from concourse.bass_utils import run_bass_kernel_spmd

# --- custom DVE op registration (idempotent) -------------------------------
import concourse.dve_ops as _dvo
from concourse.dve_spec import (
    Spec as _Spec, Src0 as _S0, Src1 as _S1, eq as _eq, Idx as _Idx,
)


def _ref_selmm(in0, in1, c0, c1, c2):
    P_ = in0.shape[0]
    x0 = in0.reshape(P_, -1).astype(np.float32)
    x1 = np.broadcast_to(in1, in0.shape).reshape(P_, -1).astype(np.float32)
    k = np.arange(x0.shape[1], dtype=np.float32)[None, :]
    return ((x1 == k) * x0).astype(np.float32)


def _register_selmm():
    name = "SEL_MASK_MUL_ANT"
    if name in _dvo._SUB_OPCODE_FOR_NAME:
        return next(op for op in _dvo.OPS if op.name == name)
    spec = _Spec(body=_eq(_S1, _Idx) * _S0, reference=_ref_selmm)
    op = _dvo.DveOp(
        name, spec, subdim=False,
        uops_sha={"v3": "8167b76bec34326c", "v4": "12842eb32a8347cf"},
    )
    row = max(_dvo._SUB_OPCODE_FOR_NAME.values()) + 1
    assert row < 0x20
    _dvo.OPS.append(op)
    _dvo._SUB_OPCODE_FOR_NAME[name] = row
    _dvo.CUSTOM_DVE_SPECS[name] = op.spec
    return op


SEL_MASK_MUL = _register_selmm()

B, N, H, W = 32, 16384, 768, 1024
NCORES = 8
BPC = B // NCORES          # batches (planes) per core = 4
P = 128
PLANE = H * W              # 786432
E = 64                     # gathered row length (f32) = 256 B
RT = PLANE // E            # rows per plane table = 12288
JJ2 = 72                   # gather slots per partition per batch
NS2 = P * JJ2              # slots per batch = 8704 (>= ~8300 measured)
U2 = NS2 // 16             # idx cols per batch = 576
J2B = 32                   # max paired-slot cols per partition
# select passes: (block_start, block_len, layer) — layer 0 = sample A,
# layer 1 = sample B (paired slots only, low cols)
PASSES = ((0, 36, 0), (36, 36, 0), (0, J2B, 1))
PASSLEN = sum(p[1] for p in PASSES)      # 100 table cols per batch
# gather chunks: (num_idxs, idx cols, G cols)
CHUNKS = [(1024, 64, 8)] * 9
BIG = 1048576.0
F32 = mybir.dt.float32
I16 = mybir.dt.int16
Alu = mybir.AluOpType
AX = mybir.AxisListType


def build(n_iters=1, init_unused=True):
    nc = bacc.Bacc(
        "TRN2", target_bir_lowering=False, debug=False,
        num_swdge_queues=4, dynamic_dma_scratch_size=32768,
    )

    img = nc.dram_tensor("img", [BPC * RT, E], F32, kind="ExternalInput")
    idx = nc.dram_tensor("idx", [P, BPC * U2], I16, kind="ExternalInput")
    aux = nc.dram_tensor("aux", [P, 2 * BPC * PASSLEN], F32,
                         kind="ExternalInput")
    out = nc.dram_tensor("out", [P, 2], F32, kind="ExternalOutput")

    with tile.TileContext(nc) as tc:
        with (
            tc.tile_pool(name="acc", bufs=1) as acc,
            tc.tile_pool(name="inp", bufs=2) as inp,
            tc.tile_pool(name="big", bufs=3) as big,
            tc.psum_pool(name="wp", bufs=1) as wp,
            tc.tile_pool(name="sm", bufs=2) as sm,
        ):
            nc.gpsimd.load_library(library_config.mlp)

            for _ in range(n_iters):
                idx16 = inp.tile([P, BPC * U2], I16, tag="idx16")
                nc.sync.dma_start(out=idx16[:], in_=idx[:, :])
                auxt = inp.tile([P, 2 * BPC * PASSLEN], F32, tag="auxt")
                nc.sync.dma_start(out=auxt[:], in_=aux[:, :])

                NP = len(PASSES)
                lc = acc.tile([P, BPC * NP], F32, tag="lc")
                cc2 = acc.tile([P, BPC * NP], F32, tag="cc2")
                gs = []
                for b in range(BPC):
                    g = big.tile([P, JJ2 * E], F32, tag="G")
                    g3 = g[:].rearrange("p (j e) -> p j e", e=E)
                    gs.append(g3)
                    uoff = b * U2
                    joff = 0
                    for k, (ci, cu, cj) in enumerate(CHUNKS):
                        nc.gpsimd.dma_gather(
                            g3[:, joff : joff + cj, :],
                            img[b * RT : (b + 1) * RT, :],
                            idx16[:, uoff : uoff + cu],
                            ci,
                            ci,
                            E,
                            single_packet=False,
                            queue_num=k % 4,
                        )
                        uoff += cu
                        joff += cj

                for b in range(BPC):
                    g3 = gs[b]
                    toff = b * PASSLEN
                    for pidx, (bs, bl, _layer) in enumerate(PASSES):
                        gt = auxt[:, toff : toff + bl]
                        ds = auxt[:, BPC * PASSLEN + toff : BPC * PASSLEN
                                  + toff + bl]
                        toff += bl

                        msel = sm.tile([P, bl], F32, tag=f"msel{pidx}")
                        nc.vector.tensor_scalar(
                            out=msel[:], in0=ds, scalar1=0.0, scalar2=None,
                            op0=Alu.is_gt,
                        )
                        w = wp.tile([P, 36 * E], F32, tag="W")
                        w3 = w[:].rearrange("p (j e) -> p j e", e=E)[
                            :, :bl, :
                        ]
                        vsel = sm.tile([P, bl], F32, tag=f"vsel{pidx}")
                        nc.vector._custom_dve(
                            SEL_MASK_MUL,
                            out=w3[:, :, :],
                            in0=g3[:, bs : bs + bl, :],
                            in1=gt.unsqueeze(2).to_broadcast([P, bl, E]),
                        )
                        nc.vector.tensor_reduce(
                            out=vsel[:], in_=w3[:, :, :], axis=AX.X,
                            op=Alu.add,
                        )
                        diff = sm.tile([P, bl], F32, tag=f"diff{pidx}")
                        nc.vector.tensor_tensor(
                            out=diff[:], in0=vsel[:], in1=ds, op=Alu.subtract
                        )
                        nc.vector.tensor_tensor(
                            out=diff[:], in0=diff[:], in1=msel[:], op=Alu.mult
                        )
                        col = b * NP + pidx
                        nc.vector.tensor_reduce(
                            out=lc[:, col : col + 1], in_=diff[:], axis=AX.X,
                            op=Alu.add, apply_absolute_value=True,
                        )
                        nc.vector.tensor_reduce(
                            out=cc2[:, col : col + 1], in_=msel[:], axis=AX.X,
                            op=Alu.add,
                        )

                losscnt = acc.tile([P, 2], F32, tag="losscnt")
                nc.vector.tensor_reduce(
                    out=losscnt[:, 0:1], in_=lc[:], axis=AX.X, op=Alu.add
                )
                nc.vector.tensor_reduce(
                    out=losscnt[:, 1:2], in_=cc2[:], axis=AX.X, op=Alu.add
                )
                nc.sync.dma_start(out=out[:, :], in_=losscnt[:])

    nc.compile()
    return nc


_NC = None


def _get_nc():
    global _NC
    if _NC is None:
        _NC = build(init_unused=False)
    return _NC


def _pack_batch(rowid, cmod, d):
    """Pack one plane's valid samples into slots of <=2 same-row samples.

    Returns idx_wrapped [16, U2] int16, gt [128, PASSLEN], dp [128, PASSLEN].
    """
    valid = np.where(d > 0)[0]
    rv = rowid[valid]
    order = np.argsort(rv, kind="stable")
    rs = rv[order]
    uniq, start, counts = np.unique(rs, return_index=True, return_counts=True)
    nsl = (counts + 1) // 2
    tot = int(nsl.sum())
    assert tot <= NS2, tot
    run = np.repeat(np.arange(uniq.size), nsl)
    cum = np.cumsum(nsl) - nsl
    mloc = np.arange(tot) - np.repeat(cum, nsl)
    ia = np.repeat(start, nsl) + 2 * mloc
    ib = ia + 1
    hasb = ib < np.repeat(start + counts, nsl)
    sA = valid[order[ia]]
    sB = np.where(hasb, valid[order[np.minimum(ib, rs.size - 1)]], -1)
    rows = uniq[run]
    # paired slots first, then singles; pads at the very end
    srt = np.argsort(~hasb, kind="stable")
    rows, sA, sB = rows[srt], sA[srt], sB[srt]
    ndbl = int(hasb.sum())
    assert ndbl <= J2B * P, ndbl

    # pads gather distinct harmless rows: same-address HBM hammering from
    # thousands of row-0 reads serializes the DMA engines
    rows_p = np.arange(NS2, dtype=np.int64) % RT
    rows_p[:tot] = rows
    sA_p = np.full(NS2, -1, np.int64)
    sA_p[:tot] = sA
    sB_p = np.full(NS2, -1, np.int64)
    sB_p[:tot] = sB
    # position i -> slot (p, jj) = (i % 128, i // 128)
    row_slot = rows_p.reshape(JJ2, P).T      # [128, JJ2]
    sA_s = sA_p.reshape(JJ2, P).T
    sB_s = sB_p.reshape(JJ2, P).T

    # wrapped idx table
    jjs = np.arange(JJ2)
    ps = np.arange(P)
    u = 64 * (jjs[None, :] // 8) + 8 * (jjs[None, :] % 8) + ps[:, None] // 16
    idxw = np.zeros((16, U2), np.int16)
    idxw[ps[:, None] % 16, u] = row_slot.astype(np.int16)

    # per-pass select/mask tables
    gt = np.empty((P, PASSLEN), np.float32)
    dp = np.empty((P, PASSLEN), np.float32)
    off = 0
    for bs, bl, layer in PASSES:
        s = (sA_s if layer == 0 else sB_s)[:, bs : bs + bl]
        ok = s >= 0
        sc = np.minimum(np.maximum(s, 0), N - 1)
        cm = np.where(ok, cmod[sc], -BIG)
        gt[:, off : off + bl] = cm + 64.0 * np.arange(bl)[None, :]
        dp[:, off : off + bl] = np.where(ok, d[sc], -1.0)
        off += bl
    return idxw, gt, dp


def make_in_maps(output, rdepth):
    in_maps = []
    for c in range(NCORES):
        sl = slice(c * BPC, (c + 1) * BPC)
        img_c = np.ascontiguousarray(
            output[sl, 0], dtype=np.float32
        ).reshape(BPC * RT, E)

        rd = np.asarray(rdepth[sl], dtype=np.float32)  # [BPC, N, 3]
        rows = rd[..., 0].astype(np.int32)
        cols = rd[..., 1].astype(np.int32)
        d = rd[..., 2]
        pix = rows * W + cols
        rowid = pix >> 6
        cmod = (pix & 63).astype(np.float32)

        idx_c = np.empty((P, BPC * U2), np.int16)
        aux_c = np.empty((P, 2 * BPC * PASSLEN), np.float32)
        for b in range(BPC):
            idxw, gt, dp = _pack_batch(rowid[b], cmod[b], d[b])
            idx_c[:, b * U2 : (b + 1) * U2] = np.tile(idxw, (8, 1))
            aux_c[:, b * PASSLEN : (b + 1) * PASSLEN] = gt
            aux_c[:, (BPC + b) * PASSLEN : (BPC + b + 1) * PASSLEN] = dp

        in_maps.append({"img": img_c, "idx": idx_c, "aux": aux_c})
    return in_maps


def combine(results):
    partials = np.stack([r["out"] for r in results])  # [8, 128, 2]
    loss = partials[..., 0].astype(np.float64).sum()
    cnt = partials[..., 1].astype(np.float64).sum()
    val = loss / max(cnt, 1.0) if cnt > 0 else 0.0
    return np.asarray(val, dtype=np.float32)


def run(output, rdepth, **kw):
    res = run_bass_kernel_spmd(
        _get_nc(), make_in_maps(output, rdepth), list(range(NCORES)), **kw
    )
    return combine(res.results), res


def kernel(output, rdepth):
    return run(output, rdepth)[0]
